# revision 30
# baseline (speedup 1.0000x reference)
"""Trainium2 Bass kernel for the ragged-sequence attention problem.

Math (per batch row):
    u      = tanh(h @ W.T + b)          h: [L, H]
    s      = u @ u_s                    masked to length, then softmax
    v      = sum_l alpha_l * h_l

Strategy (v6 — v4 + snake schedule + per-ROW exp/mul/reduce):
  - Length-aware schedule: global sort by tile count, snake-assigned to
    cores, so the shared T_seq (max over cores per rank) wastes only ~3%
    of tiles and per-core loads stay balanced.
  - h is pre-cast to bf16, padded to 256 channels, and pre-transposed on
    the host into the exact SBUF layout; channel 255 is ONES (carries the
    bias through the u-matmul and makes the softmax denominator fall out
    of the v-reduction for free).  Rows at l >= len are zero everywhere,
    so masking costs nothing on device.
  - u-matmul: 4 matmuls per 512-l group (2 k-chunks x 2 m-chunks), one
    bias-free tanh ACT over both m-chunks of a 2-bank PSUM tile.
  - scores: u_s replicated to 128 stationary columns on the host, so the
    scores matmul emits 128 IDENTICAL score rows into PSUM; the Exp ACT
    then yields the alpha BROADCAST in bf16 directly.
  - Per-ROW consumers: since len <= 2048 = 4 groups, ALL of a row's
    scores fit one 4-bank PSUM quad tile, so exp runs once per row
    (32 ACT ops instead of 83), and v is one broadcast tensor_mul (bf16
    2x mode) + one tensor_reduce straight into the resident vfin slice
    (64 DVE ops instead of ~200; the v4 trace showed ACT 79%/DVE 83%
    busy with per-op overheads the dominant reducible term).
  - 8 shortest rows are processed first so their small h tiles give the
    engines early work while the 1MB long-row DMAs stream in behind.
  - Software pipeline over (row, group) units: scores lag u by 1 unit,
    exp fires 2 units after a row's last group, mul+reduce 3 units after.
"""

import sys

import numpy as np

sys.path.insert(0, "/opt/trn_rl_repo")

import ml_dtypes  # noqa: E402

import concourse.bass as bass  # noqa: E402
import concourse.mybir as mybir  # noqa: E402
import concourse.tile as tile  # noqa: E402
from concourse.bass_utils import run_bass_kernel_spmd  # noqa: E402
import bass_rust as _br  # noqa: E402

N_CORES = 8
B, L, H = 256, 2048, 240
BPC = B // N_CORES        # batch rows per core
HP = 256                  # h channels padded (two 128 chunks)
H1 = H - 128              # 112 real channels in chunk 1
GSZ = 512
F32 = mybir.dt.float32
BF16 = mybir.dt.bfloat16
AF = mybir.ActivationFunctionType
ALU = mybir.AluOpType
AX = mybir.AxisListType
BF16NP = ml_dtypes.bfloat16

_MAXW = 1  # sync waits kept on an instruction; the rest move to nops


class _TC(tile.TileContext):
    """Walrus in this container caps sync-wait commands per instruction
    ("Too many sync wait commands"), but Tile freely attaches one wait per
    producer semaphore.  After scheduling, hoist excess waits onto dedicated
    single-wait nops inserted just before the instruction on its engine."""

    def schedule_and_allocate(self, validate_deps=False):
        ret = super().schedule_and_allocate(validate_deps)
        self._split_excess_waits()
        return ret

    def _split_excess_waits(self):
        nc = self.nc
        n_split = 0
        for fn in nc.m.functions:
            for bb in fn.blocks:
                insts = bb.instructions
                i = 0
                while i < len(insts):
                    inst = insts[i]
                    si = getattr(inst, "sync_info", None)
                    waits = list(si.on_wait) if si is not None else []
                    if len(waits) > _MAXW:
                        si.on_wait = waits[-_MAXW:]
                        inst.sync_info = si
                        for w in waits[:-_MAXW]:
                            nop = mybir.InstNoOp(
                                name=f"waitsplit-{n_split}", ins=[], outs=[])
                            n_split += 1
                            nop.engine = inst.engine
                            nop.sync_info = _br.SyncInfo(
                                on_wait=[w], on_update=[])
                            nc.register_instruction(nop, overwrite=True)
                            insts.insert(i, nop)
                            i += 1
                    i += 1


def _schedule(lens):
    """Snake-pack rows onto cores: global sort by tile count descending,
    assign ranks in snake order (0..7, 7..0, ...).  Rank i's cross-core max
    is then the (8i)-th order statistic, so the shared T_seq wastes only
    ~3% of tiles (vs ~10% for load-only LPT), while per-core loads stay
    balanced.  Returns (perm [8][32] row ids, T_seq [32])."""
    lens = np.asarray(lens).astype(np.int64)
    T = np.ceil(lens / 16).astype(np.int64)   # 16-l sub-tiles
    order = np.argsort(-T, kind="stable")
    perm = [[] for _ in range(N_CORES)]
    for i in range(BPC):
        blk = order[N_CORES * i:N_CORES * (i + 1)]
        if i % 2 == 1:
            blk = blk[::-1]
        for c in range(N_CORES):
            perm[c].append(int(blk[c]))
    T_seq = [max(int(T[perm[c][i]]) for c in range(N_CORES))
             for i in range(BPC)]
    return perm, tuple(T_seq)


def build(T_seq):
    nc = bass.Bass("TRN2", target_bir_lowering=False, debug=False,
                   num_devices=N_CORES)
    offs = []
    tot = 0
    for t in T_seq:
        offs.append(tot)
        tot += 2 * 16 * t
    h_d = nc.declare_dram_parameter("hT", [128, tot], BF16, isOutput=False)
    # processing order: 6 short rows for warmup, the long rows, then the
    # two tiniest rows last so the drain chain (scores->exp->mul->reduce)
    # after the final tanh is as short as possible
    slot_order = [29, 28, 27, 26, 25, 24] + list(range(24)) + [30, 31]
    s_first = slot_order[0]
    Wf = 2 * 16 * T_seq[s_first]
    # startup blob: one well-formed DMA (~1.9KB/partition-row) carrying
    # everything the first matmul+scores need — the v6.2 trace showed 5
    # separate small-segment DMAs costing ~9us before the first matmul
    b_d = nc.declare_dram_parameter("blob", [128, 2 * HP + 256 + Wf], BF16,
                                    isOutput=False)
    u0_d = nc.declare_dram_parameter("usr0", [128, BPC * 128], BF16,
                                     isOutput=False)
    u1_d = nc.declare_dram_parameter("usr1", [128, BPC * 128], BF16,
                                     isOutput=False)
    h0_d = nc.declare_dram_parameter("uh0", [128, 16 * 128], BF16,
                                     isOutput=False)
    h1_d = nc.declare_dram_parameter("uh1", [128, 16 * 128], BF16,
                                     isOutput=False)
    ov_d = nc.declare_dram_parameter("ov", [128, 2 * BPC], F32, isOutput=True)

    with _TC(nc) as tc:
        with (
            tc.tile_pool(name="consts", bufs=1) as cp,
            tc.tile_pool(name="ht", bufs=1) as htp,
            tc.tile_pool(name="ut", bufs=6) as utp,
            tc.tile_pool(name="ab", bufs=2) as abp,
            # DVE intermediates are single-buffered: the engine is in-order,
            # so fold1(j) always drains prod(j) before mul(j+1) rewrites it
            tc.tile_pool(name="pr", bufs=1) as prp,
            tc.tile_pool(name="fo1", bufs=1) as fo1p,
            tc.tile_pool(name="fo2", bufs=1) as fo2p,
            tc.tile_pool(name="pu", bufs=2, space="PSUM") as pup,
            tc.tile_pool(name="sg", bufs=1, space="PSUM") as sgp,
        ):
            blob = cp.tile([128, 2 * HP + 256 + Wf], BF16)
            usr0 = cp.tile([128, BPC * 128], BF16)
            usr1 = cp.tile([128, BPC * 128], BF16)
            vfin = cp.tile([128, 2 * BPC], F32)
            uh0 = cp.tile([128, 16 * 128], BF16)
            uh1 = cp.tile([128, 16 * 128], BF16)
            wtb0 = blob[:][:, 0:HP]
            wtb1 = blob[:][:, HP:2 * HP]
            sh_lo = offs[24]                # shorts are slots 24..31
            hts = cp.tile([128, tot - sh_lo], BF16)
            # 1. the startup blob: weights + first row's u_s column + its h
            nc.sync.dma_start(blob[:], b_d.ap()[:, :])
            # 2. warmup rows 24..28 in one transfer
            nc.sync.dma_start(hts[:, 0:offs[29] - sh_lo],
                              h_d.ap()[:, sh_lo:offs[29]])
            # 3. u_s head columns for processed slots 1..15
            nc.sync.dma_start(uh0[:, 128:], h0_d.ap()[:, 128:])
            nc.sync.dma_start(uh1[:, 128:], h1_d.ap()[:, 128:])
            nc.gpsimd.dma_start(usr0[:], u0_d.ap()[:, :])
            nc.gpsimd.dma_start(usr1[:], u1_d.ap()[:, :])

            class Row:
                pass

            def new_row(i, T):
                # exact-sized per-row view; ALL rows resident at once.
                # h arrives HOST-PRE-TRANSPOSED in the exact SBUF layout,
                # so each load is one plain contiguous DMA (~358 GB/s).
                # The 8 short rows live in the combined hts tile DMA'd
                # above; long rows get their own tile + DMA here.
                r = Row()
                r.i, r.T = i, T
                r.G = (16 * T + GSZ - 1) // GSZ
                r.L2 = 16 * T
                W = 2 * 16 * T
                if i == s_first:
                    r.ht = blob[:][:, 2 * HP + 256:2 * HP + 256 + W]
                elif i >= 24:
                    r.ht = hts[:][:, offs[i] - sh_lo:offs[i] - sh_lo + W]
                else:
                    tl = htp.tile([128, W], BF16, tag=f"ht{i}")
                    nc.sync.dma_start(tl[:],
                                      h_d.ap()[:, offs[i]:offs[i] + W])
                    r.ht = tl[:]
                r.ut = {}
                return r

            def nsz(r, g):
                return min(GSZ, 16 * r.T - g * GSZ)

            def emit_u(r, g):
                N = nsz(r, g)
                gs = slice(g * GSZ, g * GSZ + N)
                gs1 = slice(r.L2 + g * GSZ, r.L2 + g * GSZ + N)
                pu = pup.tile([128, 2 * GSZ], F32, tag="pu")
                # both chunk0-reading matmuls first: the u-pipeline starts
                # as soon as chunk0's data lands, while chunk1 streams
                nc.tensor.matmul(pu[:, 0:N], wtb0[:, 0:128], r.ht[:, gs],
                                 start=True, stop=False)
                nc.tensor.matmul(pu[:, GSZ:GSZ + N], wtb0[:, 128:HP],
                                 r.ht[:, gs], start=True, stop=False)
                nc.tensor.matmul(pu[:, 0:N], wtb1[:, 0:128],
                                 r.ht[:, gs1], start=False, stop=True)
                nc.tensor.matmul(pu[:, GSZ:GSZ + N], wtb1[:, 128:HP],
                                 r.ht[:, gs1], start=False, stop=True)
                ut = utp.tile([128, 2 * GSZ], BF16, tag="ut")
                nc.scalar.activation(
                    ut[:].rearrange("p (k l) -> p k l", k=2)[:, :, 0:N],
                    pu[:].rearrange("p (k l) -> p k l", k=2)[:, :, 0:N],
                    AF.Tanh)
                r.ut[g] = ut

            def emit_scores(r, g):
                # all of a row's scores land in ONE 4-bank PSUM quad tile
                # (len <= 2048 = 4 groups), so exp/mul/reduce run once per
                # ROW: 3 big ops instead of ~10 small ones (the per-op
                # overheads on ACT/DVE are what the v4 trace showed
                # dominating).  sg is single-buffered; row j+1's score
                # matmuls wait for exp(j)'s read, absorbed by PE slack.
                N = nsz(r, g)
                ut = r.ut.pop(g)
                if g == 0:
                    sgq = sgp.tile([128, 4 * GSZ], F32, tag="sg")
                    r.sgq = sgq
                sg = r.sgq
                off = g * GSZ
                if r.pidx == 0:
                    s0 = blob[:][:, 2 * HP:2 * HP + 128]
                    s1 = blob[:][:, 2 * HP + 128:2 * HP + 256]
                elif r.pidx < 16:
                    s0 = uh0[:, 128 * r.pidx:128 * r.pidx + 128]
                    s1 = uh1[:, 128 * r.pidx:128 * r.pidx + 128]
                else:
                    s0 = usr0[:, 128 * r.i:128 * r.i + 128]
                    s1 = usr1[:, 128 * r.i:128 * r.i + 128]
                nc.tensor.matmul(sg[:, off:off + N], s0,
                                 ut[:, 0:N], start=True, stop=False)
                nc.tensor.matmul(sg[:, off:off + N], s1,
                                 ut[:, GSZ:GSZ + N],
                                 start=False, stop=True)

            def emit_exp(r):
                sg = r.sgq
                ab = abp.tile([128, 4 * GSZ], BF16, tag="ab")
                nc.scalar.activation(ab[:, 0:r.L2], sg[:, 0:r.L2], AF.Exp)
                r.abt = ab

            def emit_v(r):
                # whole-row v: broadcast mul (bf16 2x), then two pairwise
                # l-fold TT-adds (also 2x) so the 1x-rate tensor_reduce
                # reads only L2/2 elements: 2.25*L2 DVE cycles/row instead
                # of 3*L2.  The folds add one bf16 rounding level to
                # partial sums of products that are already bf16 — noise.
                L2 = r.L2
                h1, q = L2 // 2, L2 // 4
                ab = r.abt
                hview = r.ht.rearrange("p (k l) -> p k l", k=2)
                prod = prp.tile([128, 2 * 4 * GSZ], BF16, tag="prod")
                pview = prod[:].rearrange("p (k l) -> p k l", k=2)
                nc.vector.tensor_mul(
                    pview[:, :, 0:L2],
                    hview,
                    ab[:, 0:L2].rearrange(
                        "p (o l) -> p o l", o=1).to_broadcast((128, 2, L2)))
                fo1 = fo1p.tile([128, 2 * 2 * GSZ], BF16, tag="fo1")
                f1v = fo1[:].rearrange("p (k l) -> p k l", k=2)
                nc.vector.tensor_add(
                    f1v[:, :, 0:h1],
                    pview[:, :, 0:h1], pview[:, :, h1:L2])
                fo2 = fo2p.tile([128, 2 * GSZ], BF16, tag="fo2")
                f2v = fo2[:].rearrange("p (k l) -> p k l", k=2)
                nc.vector.tensor_add(
                    f2v[:, :, 0:q],
                    f1v[:, :, 0:q], f1v[:, :, q:h1])
                nc.vector.tensor_reduce(
                    vfin[:, 2 * r.i:2 * r.i + 2],
                    f2v[:, :, 0:q], AX.X, ALU.add)

            # ---- software-pipelined emission over (row, group) units ----
            stream = []
            for i in slot_order:
                for g in range((16 * T_seq[i] + GSZ - 1) // GSZ):
                    stream.append((i, g))
            rows = {}
            for j, i in enumerate(slot_order):
                rows[i] = new_row(i, T_seq[i])
                rows[i].pidx = j
            # the two tail rows' h (slots 30/31, inside hts) aren't needed
            # until the very end — load them after everything else
            nc.sync.dma_start(hts[:, offs[30] - sh_lo:],
                              h_d.ap()[:, offs[30]:tot])
            # stream position -> row slot completing there
            row_done = {}
            for k, (i, g) in enumerate(stream):
                if g == rows[i].G - 1:
                    row_done[k] = i
            # SLAG=4: the single-buffered sg quad means row j+1's first
            # score matmul waits for exp(j); with scores 4 units behind
            # the u-matmuls the in-order PE queue has 4 u-matmul groups
            # to chew through while exp(j) completes, so it never stalls
            # (v6 with SLAG=1 lost ~16us to this WAR every row).
            SLAG, ELAG, VLAG = 4, 5, 6
            for k in range(len(stream) + VLAG):
                # exp first: its deps (score matmuls, 1 step back on the
                # PE queue) clear earliest, and row j+1's score matmuls
                # wait on it (single sg buffer) — don't queue it behind
                # this step's tanh on the strict-FIFO ACT queue
                if k - ELAG in row_done:
                    emit_exp(rows[row_done[k - ELAG]])
                if 0 <= k < len(stream):
                    i, g = stream[k]
                    emit_u(rows[i], g)
                if 0 <= k - SLAG < len(stream):
                    i, g = stream[k - SLAG]
                    emit_scores(rows[i], g)
                if k - VLAG in row_done:
                    emit_v(rows[row_done[k - VLAG]])
            nc.sync.dma_start(ov_d.ap()[:, :], vfin[:])

    return nc


_NC_CACHE = {}


def _get_nc(T_seq):
    if T_seq not in _NC_CACHE:
        _NC_CACHE[T_seq] = build(T_seq)
    return _NC_CACHE[T_seq]


def _prep_in_maps(short_perference, current_perference, W, bvec, length_input,
                  perm, T_seq):
    h = np.asarray(short_perference, dtype=np.float32)[0]      # [B, L, H]
    us = np.asarray(current_perference, dtype=np.float32)[0]   # [B, H]
    W = np.asarray(W, dtype=np.float32)
    bvec = np.asarray(bvec, dtype=np.float32)
    lens = np.asarray(length_input).astype(np.int64)

    wt = np.zeros((HP, HP), dtype=np.float32)                  # [c, o]
    wt[:H, :H] = W.T
    wt[HP - 1, :H] = bvec                                      # bias row
    wtb0 = wt[0:128].astype(BF16NP)
    wtb1 = wt[128:HP].astype(BF16NP)

    offs = []
    tot = 0
    for t in T_seq:
        offs.append(tot)
        tot += 2 * 16 * t
    in_maps = []
    for c in range(N_CORES):
        rows = perm[c]
        # host-side pre-transpose into the exact SBUF ht layout: per slot,
        # chunk0 [c 0:128, l] then chunk1 [c 128:256, l].  h rows at
        # l >= len are ZERO (incl. the ones/bias channel) so they
        # contribute exactly 0 to scores, numerator, and denominator —
        # the length mask costs nothing on device.
        hTc = np.zeros((128, tot), dtype=BF16NP)
        for i, r in enumerate(rows):
            n = int(lens[r])
            Lr = 16 * T_seq[i]
            tmp = np.zeros((Lr, HP), dtype=BF16NP)
            tmp[0:n, 0:H] = h[r, 0:n].astype(BF16NP)
            tmp[0:n, HP - 1] = BF16NP(1.0)
            o = offs[i]
            hTc[:, o:o + Lr] = tmp[:, 0:128].T
            hTc[:, o + Lr:o + 2 * Lr] = tmp[:, 128:HP].T
        usc = np.zeros((HP, BPC), dtype=np.float32)
        usc[0:H, :] = us[rows].T
        usr0 = np.repeat(usc[0:128].astype(BF16NP), 128, axis=1)
        usr1 = np.repeat(usc[128:HP].astype(BF16NP), 128, axis=1)
        # first 16 processed slots (6 warmup shorts, then the longest 10)
        head = [29, 28, 27, 26, 25, 24] + list(range(10))
        uh0 = np.concatenate([usr0[:, 128 * s:128 * s + 128] for s in head],
                             axis=1)
        uh1 = np.concatenate([usr1[:, 128 * s:128 * s + 128] for s in head],
                             axis=1)
        s_first = head[0]
        blob = np.concatenate(
            [wtb0, wtb1, uh0[:, 0:128], uh1[:, 0:128],
             hTc[:, offs[s_first]:offs[s_first] + 2 * 16 * T_seq[s_first]]],
            axis=1)
        in_maps.append({
            "hT": np.ascontiguousarray(hTc),
            "blob": np.ascontiguousarray(blob),
            "usr0": np.ascontiguousarray(usr0),
            "usr1": np.ascontiguousarray(usr1),
            "uh0": np.ascontiguousarray(uh0),
            "uh1": np.ascontiguousarray(uh1),
        })
    return in_maps


def run(short_perference, current_perference, W, b, length_input,
        trace=False, **run_kwargs):
    lens = np.asarray(length_input).astype(np.int64)
    perm, T_seq = _schedule(lens)
    nc = _get_nc(T_seq)
    in_maps = _prep_in_maps(short_perference, current_perference, W, b,
                            lens, perm, T_seq)
    res = run_bass_kernel_spmd(nc, in_maps, list(range(N_CORES)),
                               trace=trace, **run_kwargs)
    v = np.zeros((B, H), dtype=np.float32)
    for c in range(N_CORES):
        ov = np.asarray(res.results[c]["ov"], dtype=np.float32)  # [128,2*BPC]
        for i, r in enumerate(perm[c]):
            denom = ov[127, 2 * i + 1]
            num = np.concatenate([ov[:, 2 * i], ov[0:H1, 2 * i + 1]])
            v[r] = num / denom
    return v, res


def kernel(short_perference, current_perference, W, b, current_batch,
           length_input):
    v, _ = run(short_perference, current_perference, W, b, length_input)
    return v.astype(np.float32)


# revision 39
# speedup vs baseline: 1.0428x; 1.0428x over previous
"""Trainium2 Bass kernel for the ragged-sequence attention problem.

Math (per batch row):
    u      = tanh(h @ W.T + b)          h: [L, H]
    s      = u @ u_s                    masked to length, then softmax
    v      = sum_l alpha_l * h_l

Strategy (v6 — v4 + snake schedule + per-ROW exp/mul/reduce):
  - Length-aware schedule: global sort by tile count, snake-assigned to
    cores, so the shared T_seq (max over cores per rank) wastes only ~3%
    of tiles and per-core loads stay balanced.
  - h is pre-cast to bf16, padded to 256 channels, and pre-transposed on
    the host into the exact SBUF layout; channel 255 is ONES (carries the
    bias through the u-matmul and makes the softmax denominator fall out
    of the v-reduction for free).  Rows at l >= len are zero everywhere,
    so masking costs nothing on device.
  - u-matmul: 4 matmuls per 512-l group (2 k-chunks x 2 m-chunks), one
    bias-free tanh ACT over both m-chunks of a 2-bank PSUM tile.
  - scores: u_s replicated to 128 stationary columns on the host, so the
    scores matmul emits 128 IDENTICAL score rows into PSUM; the Exp ACT
    then yields the alpha BROADCAST in bf16 directly.
  - Per-ROW consumers: since len <= 2048 = 4 groups, ALL of a row's
    scores fit one 4-bank PSUM quad tile, so exp runs once per row
    (32 ACT ops instead of 83), and v is one broadcast tensor_mul (bf16
    2x mode) + one tensor_reduce straight into the resident vfin slice
    (64 DVE ops instead of ~200; the v4 trace showed ACT 79%/DVE 83%
    busy with per-op overheads the dominant reducible term).
  - 8 shortest rows are processed first so their small h tiles give the
    engines early work while the 1MB long-row DMAs stream in behind.
  - Software pipeline over (row, group) units: scores lag u by 1 unit,
    exp fires 2 units after a row's last group, mul+reduce 3 units after.
"""

import sys

import numpy as np

sys.path.insert(0, "/opt/trn_rl_repo")

import ml_dtypes  # noqa: E402

import concourse.bass as bass  # noqa: E402
import concourse.mybir as mybir  # noqa: E402
import concourse.tile as tile  # noqa: E402
from concourse.bass_utils import run_bass_kernel_spmd  # noqa: E402
import bass_rust as _br  # noqa: E402

N_CORES = 8
B, L, H = 256, 2048, 240
BPC = B // N_CORES        # batch rows per core
HP = 256                  # h channels padded (two 128 chunks)
H1 = H - 128              # 112 real channels in chunk 1
GSZ = 512
F32 = mybir.dt.float32
BF16 = mybir.dt.bfloat16
AF = mybir.ActivationFunctionType
ALU = mybir.AluOpType
AX = mybir.AxisListType
BF16NP = ml_dtypes.bfloat16

_MAXW = 1  # sync waits kept on an instruction; the rest move to nops


class _TC(tile.TileContext):
    """Walrus in this container caps sync-wait commands per instruction
    ("Too many sync wait commands"), but Tile freely attaches one wait per
    producer semaphore.  After scheduling, hoist excess waits onto dedicated
    single-wait nops inserted just before the instruction on its engine."""

    def schedule_and_allocate(self, validate_deps=False):
        ret = super().schedule_and_allocate(validate_deps)
        self._split_excess_waits()
        return ret

    def _split_excess_waits(self):
        nc = self.nc
        n_split = 0
        for fn in nc.m.functions:
            for bb in fn.blocks:
                insts = bb.instructions
                i = 0
                while i < len(insts):
                    inst = insts[i]
                    si = getattr(inst, "sync_info", None)
                    waits = list(si.on_wait) if si is not None else []
                    if len(waits) > _MAXW:
                        si.on_wait = waits[-_MAXW:]
                        inst.sync_info = si
                        for w in waits[:-_MAXW]:
                            nop = mybir.InstNoOp(
                                name=f"waitsplit-{n_split}", ins=[], outs=[])
                            n_split += 1
                            nop.engine = inst.engine
                            nop.sync_info = _br.SyncInfo(
                                on_wait=[w], on_update=[])
                            nc.register_instruction(nop, overwrite=True)
                            insts.insert(i, nop)
                            i += 1
                    i += 1


def _schedule(lens):
    """Snake-pack rows onto cores: global sort by tile count descending,
    assign ranks in snake order (0..7, 7..0, ...).  Rank i's cross-core max
    is then the (8i)-th order statistic, so the shared T_seq wastes only
    ~3% of tiles (vs ~10% for load-only LPT), while per-core loads stay
    balanced.  Returns (perm [8][32] row ids, T_seq [32])."""
    lens = np.asarray(lens).astype(np.int64)
    T = np.ceil(lens / 16).astype(np.int64)   # 16-l sub-tiles
    order = np.argsort(-T, kind="stable")
    perm = [[] for _ in range(N_CORES)]
    for i in range(BPC):
        blk = order[N_CORES * i:N_CORES * (i + 1)]
        if i % 2 == 1:
            blk = blk[::-1]
        for c in range(N_CORES):
            perm[c].append(int(blk[c]))
    T_seq = [max(int(T[perm[c][i]]) for c in range(N_CORES))
             for i in range(BPC)]
    return perm, tuple(T_seq)


def build(T_seq):
    nc = bass.Bass("TRN2", target_bir_lowering=False, debug=False,
                   num_devices=N_CORES)
    offs = []
    tot = 0
    for t in T_seq:
        offs.append(tot)
        tot += 2 * 16 * t
    h_d = nc.declare_dram_parameter("hT", [128, tot], BF16, isOutput=False)
    # processing order: 6 short rows for warmup, the long rows, then the
    # two tiniest rows last so the drain chain (scores->exp->mul->reduce)
    # after the final tanh is as short as possible
    slot_order = [29, 28, 27, 26, 25, 24] + list(range(24)) + [30, 31]
    s_first = slot_order[0]
    Wf = 2 * 16 * T_seq[s_first]
    # startup blob: one well-formed DMA (~1.3KB/partition-row) carrying
    # everything the first row needs: both W chunks, the per-slot u_s
    # tables (scores use a stride-0 broadcast stationary straight off the
    # [128, 1] u_s column — verified exact on HW — so no replicated
    # tables are ever DMA'd), and the first row's h
    b_d = nc.declare_dram_parameter("blob", [128, 2 * HP + 2 * BPC + Wf],
                                    BF16, isOutput=False)
    ov_d = nc.declare_dram_parameter("ov", [128, 2 * BPC], F32, isOutput=True)

    with _TC(nc) as tc:
        with (
            tc.tile_pool(name="consts", bufs=1) as cp,
            tc.tile_pool(name="ht", bufs=1) as htp,
            tc.tile_pool(name="ut", bufs=6) as utp,
            tc.tile_pool(name="ab", bufs=2) as abp,
            # DVE intermediates are single-buffered: the engine is in-order,
            # so fold1(j) always drains prod(j) before mul(j+1) rewrites it
            tc.tile_pool(name="pr", bufs=1) as prp,
            tc.tile_pool(name="fo1", bufs=1) as fo1p,
            tc.tile_pool(name="fo2", bufs=1) as fo2p,
            tc.tile_pool(name="pu", bufs=2, space="PSUM") as pup,
            tc.tile_pool(name="sg", bufs=1, space="PSUM") as sgp,
        ):
            blob = cp.tile([128, 2 * HP + 2 * BPC + Wf], BF16)
            vfin = cp.tile([128, 2 * BPC], F32)
            wtb0 = blob[:][:, 0:HP]
            wtb1 = blob[:][:, HP:2 * HP]
            sh_lo = offs[24]                # shorts are slots 24..31
            hts = cp.tile([128, tot - sh_lo], BF16)
            # 1. the startup blob (first row's everything)
            nc.sync.dma_start(blob[:], b_d.ap()[:, :])
            # 2. warmup rows in two pieces so compute unblocks row by row
            nc.sync.dma_start(hts[:, offs[27] - sh_lo:offs[29] - sh_lo],
                              h_d.ap()[:, offs[27]:offs[29]])
            nc.sync.dma_start(hts[:, 0:offs[27] - sh_lo],
                              h_d.ap()[:, sh_lo:offs[27]])

            class Row:
                pass

            def new_row(i, T):
                # exact-sized per-row view; ALL rows resident at once.
                # h arrives HOST-PRE-TRANSPOSED in the exact SBUF layout,
                # so each load is one plain contiguous DMA (~358 GB/s).
                # The 8 short rows live in the combined hts tile DMA'd
                # above; long rows get their own tile + DMA here.
                r = Row()
                r.i, r.T = i, T
                r.G = (16 * T + GSZ - 1) // GSZ
                r.L2 = 16 * T
                W = 2 * 16 * T
                if i == s_first:
                    base = 2 * HP + 2 * BPC
                    r.ht = blob[:][:, base:base + W]
                elif i >= 24:
                    r.ht = hts[:][:, offs[i] - sh_lo:offs[i] - sh_lo + W]
                else:
                    tl = htp.tile([128, W], BF16, tag=f"ht{i}")
                    nc.sync.dma_start(tl[:],
                                      h_d.ap()[:, offs[i]:offs[i] + W])
                    r.ht = tl[:]
                r.ut = {}
                return r

            def nsz(r, g):
                return min(GSZ, 16 * r.T - g * GSZ)

            def emit_u(r, g):
                N = nsz(r, g)
                gs = slice(g * GSZ, g * GSZ + N)
                gs1 = slice(r.L2 + g * GSZ, r.L2 + g * GSZ + N)
                pu = pup.tile([128, 2 * GSZ], F32, tag="pu")
                # both chunk0-reading matmuls first: the u-pipeline starts
                # as soon as chunk0's data lands, while chunk1 streams
                nc.tensor.matmul(pu[:, 0:N], wtb0[:, 0:128], r.ht[:, gs],
                                 start=True, stop=False)
                nc.tensor.matmul(pu[:, GSZ:GSZ + N], wtb0[:, 128:HP],
                                 r.ht[:, gs], start=True, stop=False)
                nc.tensor.matmul(pu[:, 0:N], wtb1[:, 0:128],
                                 r.ht[:, gs1], start=False, stop=True)
                nc.tensor.matmul(pu[:, GSZ:GSZ + N], wtb1[:, 128:HP],
                                 r.ht[:, gs1], start=False, stop=True)
                ut = utp.tile([128, 2 * GSZ], BF16, tag="ut")
                if N == GSZ:
                    # full group: the [128,2,N] view is contiguous — emit a
                    # plain 2D AP (3D APs cost ~+150ns/op on ACT, measured)
                    nc.scalar.activation(ut[:, 0:2 * GSZ], pu[:, 0:2 * GSZ],
                                         AF.Tanh)
                else:
                    nc.scalar.activation(
                        ut[:].rearrange("p (k l) -> p k l", k=2)[:, :, 0:N],
                        pu[:].rearrange("p (k l) -> p k l", k=2)[:, :, 0:N],
                        AF.Tanh)
                r.ut[g] = ut

            def emit_scores(r, g):
                # all of a row's scores land in ONE 4-bank PSUM quad tile
                # (len <= 2048 = 4 groups), so exp/mul/reduce run once per
                # ROW: 3 big ops instead of ~10 small ones (the per-op
                # overheads on ACT/DVE are what the v4 trace showed
                # dominating).  sg is single-buffered; row j+1's score
                # matmuls wait for exp(j)'s read, absorbed by PE slack.
                N = nsz(r, g)
                ut = r.ut.pop(g)
                if g == 0:
                    sgq = sgp.tile([128, 4 * GSZ], F32, tag="sg")
                    r.sgq = sgq
                sg = r.sgq
                off = g * GSZ
                # stationary = this slot's u_s column broadcast to 128
                # identical columns via a stride-0 AP (PSUM rows come out
                # identical, giving the alpha broadcast for free)
                s0 = blob[:][:, 2 * HP + r.i:2 * HP + r.i + 1]\
                    .broadcast_to((128, 128))
                s1 = blob[:][:, 2 * HP + BPC + r.i:2 * HP + BPC + r.i + 1]\
                    .broadcast_to((128, 128))
                nc.tensor.matmul(sg[:, off:off + N], s0,
                                 ut[:, 0:N], start=True, stop=False)
                nc.tensor.matmul(sg[:, off:off + N], s1,
                                 ut[:, GSZ:GSZ + N],
                                 start=False, stop=True)

            def emit_exp(r):
                sg = r.sgq
                ab = abp.tile([128, 4 * GSZ], BF16, tag="ab")
                nc.scalar.activation(ab[:, 0:r.L2], sg[:, 0:r.L2], AF.Exp)
                r.abt = ab

            def emit_v(r):
                # whole-row v: broadcast mul (bf16 2x), then two pairwise
                # l-fold TT-adds (also 2x) so the 1x-rate tensor_reduce
                # reads only L2/2 elements: 2.25*L2 DVE cycles/row instead
                # of 3*L2.  The folds add one bf16 rounding level to
                # partial sums of products that are already bf16 — noise.
                L2 = r.L2
                h1, q = L2 // 2, L2 // 4
                ab = r.abt
                hview = r.ht.rearrange("p (k l) -> p k l", k=2)
                prod = prp.tile([128, 2 * 4 * GSZ], BF16, tag="prod")
                pview = prod[:].rearrange(
                    "p (k l) -> p k l", k=2)[:, :, 0:L2]
                nc.vector.tensor_mul(
                    pview[:, :, 0:L2],
                    hview,
                    ab[:, 0:L2].rearrange(
                        "p (o l) -> p o l", o=1).to_broadcast((128, 2, L2)))
                fo1 = fo1p.tile([128, 2 * 2 * GSZ], BF16, tag="fo1")
                f1v = fo1[:].rearrange("p (k l) -> p k l", k=2)
                nc.vector.tensor_add(
                    f1v[:, :, 0:h1],
                    pview[:, :, 0:h1], pview[:, :, h1:L2])
                fo2 = fo2p.tile([128, 2 * GSZ], BF16, tag="fo2")
                f2v = fo2[:].rearrange("p (k l) -> p k l", k=2)
                nc.vector.tensor_add(
                    f2v[:, :, 0:q],
                    f1v[:, :, 0:q], f1v[:, :, q:h1])
                nc.vector.tensor_reduce(
                    vfin[:, 2 * r.i:2 * r.i + 2],
                    f2v[:, :, 0:q], AX.X, ALU.add)

            # ---- software-pipelined emission over (row, group) units ----
            stream = []
            for i in slot_order:
                for g in range((16 * T_seq[i] + GSZ - 1) // GSZ):
                    stream.append((i, g))
            rows = {}
            for i in slot_order:
                rows[i] = new_row(i, T_seq[i])
            # the two tail rows' h (slots 30/31, inside hts) aren't needed
            # until the very end — load them after everything else
            nc.sync.dma_start(hts[:, offs[30] - sh_lo:],
                              h_d.ap()[:, offs[30]:tot])
            # stream position -> row slot completing there
            row_done = {}
            for k, (i, g) in enumerate(stream):
                if g == rows[i].G - 1:
                    row_done[k] = i
            # SLAG=4: the single-buffered sg quad means row j+1's first
            # score matmul waits for exp(j); with scores 4 units behind
            # the u-matmuls the in-order PE queue has 4 u-matmul groups
            # to chew through while exp(j) completes, so it never stalls
            # (v6 with SLAG=1 lost ~16us to this WAR every row).
            SLAG, ELAG, VLAG = 4, 5, 6
            for k in range(len(stream) + VLAG):
                # exp first: its deps (score matmuls, 1 step back on the
                # PE queue) clear earliest, and row j+1's score matmuls
                # wait on it (single sg buffer) — don't queue it behind
                # this step's tanh on the strict-FIFO ACT queue
                if k - ELAG in row_done:
                    emit_exp(rows[row_done[k - ELAG]])
                if 0 <= k < len(stream):
                    i, g = stream[k]
                    emit_u(rows[i], g)
                if 0 <= k - SLAG < len(stream):
                    i, g = stream[k - SLAG]
                    emit_scores(rows[i], g)
                if k - VLAG in row_done:
                    emit_v(rows[row_done[k - VLAG]])
            nc.sync.dma_start(ov_d.ap()[:, :], vfin[:])

    return nc


_NC_CACHE = {}


def _get_nc(T_seq):
    if T_seq not in _NC_CACHE:
        _NC_CACHE[T_seq] = build(T_seq)
    return _NC_CACHE[T_seq]


def _prep_in_maps(short_perference, current_perference, W, bvec, length_input,
                  perm, T_seq):
    h = np.asarray(short_perference, dtype=np.float32)[0]      # [B, L, H]
    us = np.asarray(current_perference, dtype=np.float32)[0]   # [B, H]
    W = np.asarray(W, dtype=np.float32)
    bvec = np.asarray(bvec, dtype=np.float32)
    lens = np.asarray(length_input).astype(np.int64)

    wt = np.zeros((HP, HP), dtype=np.float32)                  # [c, o]
    wt[:H, :H] = W.T
    wt[HP - 1, :H] = bvec                                      # bias row
    wtb0 = wt[0:128].astype(BF16NP)
    wtb1 = wt[128:HP].astype(BF16NP)

    offs = []
    tot = 0
    for t in T_seq:
        offs.append(tot)
        tot += 2 * 16 * t
    in_maps = []
    for c in range(N_CORES):
        rows = perm[c]
        # host-side pre-transpose into the exact SBUF ht layout: per slot,
        # chunk0 [c 0:128, l] then chunk1 [c 128:256, l].  h rows at
        # l >= len are ZERO (incl. the ones/bias channel) so they
        # contribute exactly 0 to scores, numerator, and denominator —
        # the length mask costs nothing on device.
        hTc = np.zeros((128, tot), dtype=BF16NP)
        for i, r in enumerate(rows):
            n = int(lens[r])
            Lr = 16 * T_seq[i]
            tmp = np.zeros((Lr, HP), dtype=BF16NP)
            tmp[0:n, 0:H] = h[r, 0:n].astype(BF16NP)
            tmp[0:n, HP - 1] = BF16NP(1.0)
            o = offs[i]
            hTc[:, o:o + Lr] = tmp[:, 0:128].T
            hTc[:, o + Lr:o + 2 * Lr] = tmp[:, 128:HP].T
        usc = np.zeros((HP, BPC), dtype=np.float32)
        usc[0:H, :] = us[rows].T
        usc0 = usc[0:128].astype(BF16NP)
        usc1 = usc[128:HP].astype(BF16NP)
        s_first = 29
        blob = np.concatenate(
            [wtb0, wtb1, usc0, usc1,
             hTc[:, offs[s_first]:offs[s_first] + 2 * 16 * T_seq[s_first]]],
            axis=1)
        in_maps.append({
            "hT": np.ascontiguousarray(hTc),
            "blob": np.ascontiguousarray(blob),
        })
    return in_maps


def run(short_perference, current_perference, W, b, length_input,
        trace=False, **run_kwargs):
    lens = np.asarray(length_input).astype(np.int64)
    perm, T_seq = _schedule(lens)
    nc = _get_nc(T_seq)
    in_maps = _prep_in_maps(short_perference, current_perference, W, b,
                            lens, perm, T_seq)
    res = run_bass_kernel_spmd(nc, in_maps, list(range(N_CORES)),
                               trace=trace, **run_kwargs)
    v = np.zeros((B, H), dtype=np.float32)
    for c in range(N_CORES):
        ov = np.asarray(res.results[c]["ov"], dtype=np.float32)  # [128,2*BPC]
        for i, r in enumerate(perm[c]):
            denom = ov[127, 2 * i + 1]
            num = np.concatenate([ov[:, 2 * i], ov[0:H1, 2 * i + 1]])
            v[r] = num / denom
    return v, res


def kernel(short_perference, current_perference, W, b, current_batch,
           length_input):
    v, _ = run(short_perference, current_perference, W, b, length_input)
    return v.astype(np.float32)


# revision 41
# speedup vs baseline: 1.0479x; 1.0049x over previous
"""Trainium2 Bass kernel for the ragged-sequence attention problem.

Math (per batch row):
    u      = tanh(h @ W.T + b)          h: [L, H]
    s      = u @ u_s                    masked to length, then softmax
    v      = sum_l alpha_l * h_l

Strategy (v6 — v4 + snake schedule + per-ROW exp/mul/reduce):
  - Length-aware schedule: global sort by tile count, snake-assigned to
    cores, so the shared T_seq (max over cores per rank) wastes only ~3%
    of tiles and per-core loads stay balanced.
  - h is pre-cast to bf16, padded to 256 channels, and pre-transposed on
    the host into the exact SBUF layout; channel 255 is ONES (carries the
    bias through the u-matmul and makes the softmax denominator fall out
    of the v-reduction for free).  Rows at l >= len are zero everywhere,
    so masking costs nothing on device.
  - u-matmul: 4 matmuls per 512-l group (2 k-chunks x 2 m-chunks), one
    bias-free tanh ACT over both m-chunks of a 2-bank PSUM tile.
  - scores: u_s replicated to 128 stationary columns on the host, so the
    scores matmul emits 128 IDENTICAL score rows into PSUM; the Exp ACT
    then yields the alpha BROADCAST in bf16 directly.
  - Per-ROW consumers: since len <= 2048 = 4 groups, ALL of a row's
    scores fit one 4-bank PSUM quad tile, so exp runs once per row
    (32 ACT ops instead of 83), and v is one broadcast tensor_mul (bf16
    2x mode) + one tensor_reduce straight into the resident vfin slice
    (64 DVE ops instead of ~200; the v4 trace showed ACT 79%/DVE 83%
    busy with per-op overheads the dominant reducible term).
  - 8 shortest rows are processed first so their small h tiles give the
    engines early work while the 1MB long-row DMAs stream in behind.
  - Software pipeline over (row, group) units: scores lag u by 1 unit,
    exp fires 2 units after a row's last group, mul+reduce 3 units after.
"""

import sys

import numpy as np

sys.path.insert(0, "/opt/trn_rl_repo")

import ml_dtypes  # noqa: E402

import concourse.bass as bass  # noqa: E402
import concourse.mybir as mybir  # noqa: E402
import concourse.tile as tile  # noqa: E402
from concourse.bass_utils import run_bass_kernel_spmd  # noqa: E402
import bass_rust as _br  # noqa: E402

N_CORES = 8
B, L, H = 256, 2048, 240
BPC = B // N_CORES        # batch rows per core
HP = 256                  # h channels padded (two 128 chunks)
H1 = H - 128              # 112 real channels in chunk 1
GSZ = 512
F32 = mybir.dt.float32
BF16 = mybir.dt.bfloat16
AF = mybir.ActivationFunctionType
ALU = mybir.AluOpType
AX = mybir.AxisListType
BF16NP = ml_dtypes.bfloat16

_MAXW = 1  # sync waits kept on an instruction; the rest move to nops


class _TC(tile.TileContext):
    """Walrus in this container caps sync-wait commands per instruction
    ("Too many sync wait commands"), but Tile freely attaches one wait per
    producer semaphore.  After scheduling, hoist excess waits onto dedicated
    single-wait nops inserted just before the instruction on its engine."""

    def schedule_and_allocate(self, validate_deps=False):
        ret = super().schedule_and_allocate(validate_deps)
        self._split_excess_waits()
        return ret

    def _split_excess_waits(self):
        nc = self.nc
        n_split = 0
        for fn in nc.m.functions:
            for bb in fn.blocks:
                insts = bb.instructions
                i = 0
                while i < len(insts):
                    inst = insts[i]
                    si = getattr(inst, "sync_info", None)
                    waits = list(si.on_wait) if si is not None else []
                    if len(waits) > _MAXW:
                        si.on_wait = waits[-_MAXW:]
                        inst.sync_info = si
                        for w in waits[:-_MAXW]:
                            nop = mybir.InstNoOp(
                                name=f"waitsplit-{n_split}", ins=[], outs=[])
                            n_split += 1
                            nop.engine = inst.engine
                            nop.sync_info = _br.SyncInfo(
                                on_wait=[w], on_update=[])
                            nc.register_instruction(nop, overwrite=True)
                            insts.insert(i, nop)
                            i += 1
                    i += 1


def _schedule(lens):
    """Snake-pack rows onto cores: global sort by tile count descending,
    assign ranks in snake order (0..7, 7..0, ...).  Rank i's cross-core max
    is then the (8i)-th order statistic, so the shared T_seq wastes only
    ~3% of tiles (vs ~10% for load-only LPT), while per-core loads stay
    balanced.  Returns (perm [8][32] row ids, T_seq [32])."""
    lens = np.asarray(lens).astype(np.int64)
    T = np.ceil(lens / 16).astype(np.int64)   # 16-l sub-tiles
    order = np.argsort(-T, kind="stable")
    perm = [[] for _ in range(N_CORES)]
    for i in range(BPC):
        blk = order[N_CORES * i:N_CORES * (i + 1)]
        if i % 2 == 1:
            blk = blk[::-1]
        for c in range(N_CORES):
            perm[c].append(int(blk[c]))
    T_seq = [max(int(T[perm[c][i]]) for c in range(N_CORES))
             for i in range(BPC)]
    return perm, tuple(T_seq)


def build(T_seq):
    nc = bass.Bass("TRN2", target_bir_lowering=False, debug=False,
                   num_devices=N_CORES)
    offs = []
    tot = 0
    for t in T_seq:
        offs.append(tot)
        tot += 2 * 16 * t
    h_d = nc.declare_dram_parameter("hT", [128, tot], BF16, isOutput=False)
    # processing order: 6 short rows for warmup, the long rows, then the
    # two tiniest rows last so the drain chain (scores->exp->mul->reduce)
    # after the final tanh is as short as possible
    slot_order = [29, 28, 27, 26, 25, 24] + list(range(24)) + [30, 31]
    s_first = slot_order[0]
    Wf = 2 * 16 * T_seq[s_first]
    # startup blob: one well-formed DMA (~1.3KB/partition-row) carrying
    # everything the first row needs: both W chunks, the per-slot u_s
    # tables (scores use a stride-0 broadcast stationary straight off the
    # [128, 1] u_s column — verified exact on HW — so no replicated
    # tables are ever DMA'd), and the first row's h
    b_d = nc.declare_dram_parameter("blob", [128, 2 * HP + 2 * BPC + Wf],
                                    BF16, isOutput=False)
    ov_d = nc.declare_dram_parameter("ov", [128, 2 * BPC], F32, isOutput=True)

    with _TC(nc) as tc:
        with (
            tc.tile_pool(name="consts", bufs=1) as cp,
            tc.tile_pool(name="ht", bufs=1) as htp,
            tc.tile_pool(name="ut", bufs=6) as utp,
            tc.tile_pool(name="ab", bufs=2) as abp,
            # DVE intermediates are single-buffered: the engine is in-order,
            # so fold1(j) always drains prod(j) before mul(j+1) rewrites it
            tc.tile_pool(name="pr", bufs=1) as prp,
            tc.tile_pool(name="fo1", bufs=1) as fo1p,
            tc.tile_pool(name="fo2", bufs=1) as fo2p,
            tc.tile_pool(name="pu", bufs=2, space="PSUM") as pup,
            tc.tile_pool(name="sg", bufs=1, space="PSUM") as sgp,
        ):
            blob = cp.tile([128, 2 * HP + 2 * BPC + Wf], BF16)
            vfin = cp.tile([128, 2 * BPC], F32)
            wtb0 = blob[:][:, 0:HP]
            wtb1 = blob[:][:, HP:2 * HP]
            sh_lo = offs[24]                # shorts are slots 24..31
            hts = cp.tile([128, tot - sh_lo], BF16)
            # 1. the startup blob (first row's everything)
            nc.sync.dma_start(blob[:], b_d.ap()[:, :])
            # 2. warmup rows in two pieces so compute unblocks row by row
            nc.sync.dma_start(hts[:, offs[27] - sh_lo:offs[29] - sh_lo],
                              h_d.ap()[:, offs[27]:offs[29]])
            nc.sync.dma_start(hts[:, 0:offs[27] - sh_lo],
                              h_d.ap()[:, sh_lo:offs[27]])

            class Row:
                pass

            def new_row(i, T):
                # exact-sized per-row view; ALL rows resident at once.
                # h arrives HOST-PRE-TRANSPOSED in the exact SBUF layout,
                # so each load is one plain contiguous DMA (~358 GB/s).
                # The 8 short rows live in the combined hts tile DMA'd
                # above; long rows get their own tile + DMA here.
                r = Row()
                r.i, r.T = i, T
                r.G = (16 * T + GSZ - 1) // GSZ
                r.L2 = 16 * T
                W = 2 * 16 * T
                if i == s_first:
                    base = 2 * HP + 2 * BPC
                    r.ht = blob[:][:, base:base + W]
                elif i >= 24:
                    r.ht = hts[:][:, offs[i] - sh_lo:offs[i] - sh_lo + W]
                else:
                    tl = htp.tile([128, W], BF16, tag=f"ht{i}")
                    nc.sync.dma_start(tl[:],
                                      h_d.ap()[:, offs[i]:offs[i] + W])
                    r.ht = tl[:]
                r.ut = {}
                return r

            def nsz(r, g):
                return min(GSZ, 16 * r.T - g * GSZ)

            def emit_u(r, g):
                N = nsz(r, g)
                gs = slice(g * GSZ, g * GSZ + N)
                gs1 = slice(r.L2 + g * GSZ, r.L2 + g * GSZ + N)
                pu = pup.tile([128, 2 * GSZ], F32, tag="pu")
                # both chunk0-reading matmuls first: the u-pipeline starts
                # as soon as chunk0's data lands, while chunk1 streams
                nc.tensor.matmul(pu[:, 0:N], wtb0[:, 0:128], r.ht[:, gs],
                                 start=True, stop=False)
                nc.tensor.matmul(pu[:, GSZ:GSZ + N], wtb0[:, 128:HP],
                                 r.ht[:, gs], start=True, stop=False)
                nc.tensor.matmul(pu[:, 0:N], wtb1[:, 0:128],
                                 r.ht[:, gs1], start=False, stop=True)
                nc.tensor.matmul(pu[:, GSZ:GSZ + N], wtb1[:, 128:HP],
                                 r.ht[:, gs1], start=False, stop=True)
                ut = utp.tile([128, 2 * GSZ], BF16, tag="ut")
                if N == GSZ:
                    # full group: the [128,2,N] view is contiguous — emit a
                    # plain 2D AP (3D APs cost ~+150ns/op on ACT, measured)
                    nc.scalar.activation(ut[:, 0:2 * GSZ], pu[:, 0:2 * GSZ],
                                         AF.Tanh)
                else:
                    nc.scalar.activation(
                        ut[:].rearrange("p (k l) -> p k l", k=2)[:, :, 0:N],
                        pu[:].rearrange("p (k l) -> p k l", k=2)[:, :, 0:N],
                        AF.Tanh)
                r.ut[g] = ut

            def emit_scores(r, g):
                # all of a row's scores land in ONE 4-bank PSUM quad tile
                # (len <= 2048 = 4 groups), so exp/mul/reduce run once per
                # ROW: 3 big ops instead of ~10 small ones (the per-op
                # overheads on ACT/DVE are what the v4 trace showed
                # dominating).  sg is single-buffered; row j+1's score
                # matmuls wait for exp(j)'s read, absorbed by PE slack.
                N = nsz(r, g)
                ut = r.ut.pop(g)
                if g == 0:
                    sgq = sgp.tile([128, 4 * GSZ], F32, tag="sg")
                    r.sgq = sgq
                sg = r.sgq
                off = g * GSZ
                # stationary = this slot's u_s column broadcast to 128
                # identical columns via a stride-0 AP (PSUM rows come out
                # identical, giving the alpha broadcast for free)
                s0 = blob[:][:, 2 * HP + r.i:2 * HP + r.i + 1]\
                    .broadcast_to((128, 128))
                s1 = blob[:][:, 2 * HP + BPC + r.i:2 * HP + BPC + r.i + 1]\
                    .broadcast_to((128, 128))
                nc.tensor.matmul(sg[:, off:off + N], s0,
                                 ut[:, 0:N], start=True, stop=False)
                nc.tensor.matmul(sg[:, off:off + N], s1,
                                 ut[:, GSZ:GSZ + N],
                                 start=False, stop=True)

            def emit_exp(r):
                sg = r.sgq
                ab = abp.tile([128, 4 * GSZ], BF16, tag="ab")
                nc.scalar.activation(ab[:, 0:r.L2], sg[:, 0:r.L2], AF.Exp)
                r.abt = ab

            def emit_v(r):
                # whole-row v: broadcast mul (bf16 2x), then two pairwise
                # l-fold TT-adds (also 2x) so the 1x-rate tensor_reduce
                # reads only L2/2 elements: 2.25*L2 DVE cycles/row instead
                # of 3*L2.  The folds add one bf16 rounding level to
                # partial sums of products that are already bf16 — noise.
                L2 = r.L2
                h1, q = L2 // 2, L2 // 4
                ab = r.abt
                hview = r.ht.rearrange("p (k l) -> p k l", k=2)
                prod = prp.tile([128, 2 * 4 * GSZ], BF16, tag="prod")
                pview = prod[:].rearrange(
                    "p (k l) -> p k l", k=2)[:, :, 0:L2]
                nc.vector.tensor_mul(
                    pview[:, :, 0:L2],
                    hview,
                    ab[:, 0:L2].rearrange(
                        "p (o l) -> p o l", o=1).to_broadcast((128, 2, L2)))
                fo1 = fo1p.tile([128, 2 * 2 * GSZ], BF16, tag="fo1")
                f1v = fo1[:].rearrange("p (k l) -> p k l", k=2)
                nc.vector.tensor_add(
                    f1v[:, :, 0:h1],
                    pview[:, :, 0:h1], pview[:, :, h1:L2])
                fo2 = fo2p.tile([128, 2 * GSZ], BF16, tag="fo2")
                f2v = fo2[:].rearrange("p (k l) -> p k l", k=2)
                nc.vector.tensor_add(
                    f2v[:, :, 0:q],
                    f1v[:, :, 0:q], f1v[:, :, q:h1])
                nc.vector.tensor_reduce(
                    vfin[:, 2 * r.i:2 * r.i + 2],
                    f2v[:, :, 0:q], AX.X, ALU.add)

            # ---- software-pipelined emission over (row, group) units ----
            stream = []
            for i in slot_order:
                for g in range((16 * T_seq[i] + GSZ - 1) // GSZ):
                    stream.append((i, g))
            rows = {}
            for i in slot_order:
                rows[i] = new_row(i, T_seq[i])
            # the two tail rows' h (slots 30/31, inside hts) aren't needed
            # until the very end — load them after everything else
            nc.sync.dma_start(hts[:, offs[30] - sh_lo:],
                              h_d.ap()[:, offs[30]:tot])
            # stream position -> row slot completing there
            row_done = {}
            for k, (i, g) in enumerate(stream):
                if g == rows[i].G - 1:
                    row_done[k] = i
            # SLAG=4: the single-buffered sg quad means row j+1's first
            # score matmul waits for exp(j); with scores 4 units behind
            # the u-matmuls the in-order PE queue has 4 u-matmul groups
            # to chew through while exp(j) completes, so it never stalls
            # (v6 with SLAG=1 lost ~16us to this WAR every row).  The lag
            # TAPERS to 1 for the last 3 units (tiny tail rows): at the
            # flush there is no more u-work to hide behind, and the v6.5
            # trace showed the bunched S/E/V chains draining ~5us serially
            # after the last tanh.
            nstream = len(stream)

            def slag(j):
                # smooth taper keeps S due-steps monotone in j
                return max(1, min(4, nstream - 1 - j))

            pend_e, pend_v = {}, {}
            s_ptr = 0
            for k in range(nstream + 4):
                for i in pend_e.pop(k, []):
                    emit_exp(rows[i])
                if 0 <= k < nstream:
                    i, g = stream[k]
                    emit_u(rows[i], g)
                while s_ptr < nstream and k >= s_ptr + slag(s_ptr):
                    i, g = stream[s_ptr]
                    emit_scores(rows[i], g)
                    if s_ptr in row_done:
                        pend_e.setdefault(k + 1, []).append(i)
                        pend_v.setdefault(k + 2, []).append(i)
                    s_ptr += 1
                for i in pend_v.pop(k, []):
                    emit_v(rows[i])
            nc.sync.dma_start(ov_d.ap()[:, :], vfin[:])

    return nc


_NC_CACHE = {}


def _get_nc(T_seq):
    if T_seq not in _NC_CACHE:
        _NC_CACHE[T_seq] = build(T_seq)
    return _NC_CACHE[T_seq]


def _prep_in_maps(short_perference, current_perference, W, bvec, length_input,
                  perm, T_seq):
    h = np.asarray(short_perference, dtype=np.float32)[0]      # [B, L, H]
    us = np.asarray(current_perference, dtype=np.float32)[0]   # [B, H]
    W = np.asarray(W, dtype=np.float32)
    bvec = np.asarray(bvec, dtype=np.float32)
    lens = np.asarray(length_input).astype(np.int64)

    wt = np.zeros((HP, HP), dtype=np.float32)                  # [c, o]
    wt[:H, :H] = W.T
    wt[HP - 1, :H] = bvec                                      # bias row
    wtb0 = wt[0:128].astype(BF16NP)
    wtb1 = wt[128:HP].astype(BF16NP)

    offs = []
    tot = 0
    for t in T_seq:
        offs.append(tot)
        tot += 2 * 16 * t
    in_maps = []
    for c in range(N_CORES):
        rows = perm[c]
        # host-side pre-transpose into the exact SBUF ht layout: per slot,
        # chunk0 [c 0:128, l] then chunk1 [c 128:256, l].  h rows at
        # l >= len are ZERO (incl. the ones/bias channel) so they
        # contribute exactly 0 to scores, numerator, and denominator —
        # the length mask costs nothing on device.
        hTc = np.zeros((128, tot), dtype=BF16NP)
        for i, r in enumerate(rows):
            n = int(lens[r])
            Lr = 16 * T_seq[i]
            tmp = np.zeros((Lr, HP), dtype=BF16NP)
            tmp[0:n, 0:H] = h[r, 0:n].astype(BF16NP)
            tmp[0:n, HP - 1] = BF16NP(1.0)
            o = offs[i]
            hTc[:, o:o + Lr] = tmp[:, 0:128].T
            hTc[:, o + Lr:o + 2 * Lr] = tmp[:, 128:HP].T
        usc = np.zeros((HP, BPC), dtype=np.float32)
        usc[0:H, :] = us[rows].T
        usc0 = usc[0:128].astype(BF16NP)
        usc1 = usc[128:HP].astype(BF16NP)
        s_first = 29
        blob = np.concatenate(
            [wtb0, wtb1, usc0, usc1,
             hTc[:, offs[s_first]:offs[s_first] + 2 * 16 * T_seq[s_first]]],
            axis=1)
        in_maps.append({
            "hT": np.ascontiguousarray(hTc),
            "blob": np.ascontiguousarray(blob),
        })
    return in_maps


def run(short_perference, current_perference, W, b, length_input,
        trace=False, **run_kwargs):
    lens = np.asarray(length_input).astype(np.int64)
    perm, T_seq = _schedule(lens)
    nc = _get_nc(T_seq)
    in_maps = _prep_in_maps(short_perference, current_perference, W, b,
                            lens, perm, T_seq)
    res = run_bass_kernel_spmd(nc, in_maps, list(range(N_CORES)),
                               trace=trace, **run_kwargs)
    v = np.zeros((B, H), dtype=np.float32)
    for c in range(N_CORES):
        ov = np.asarray(res.results[c]["ov"], dtype=np.float32)  # [128,2*BPC]
        for i, r in enumerate(perm[c]):
            denom = ov[127, 2 * i + 1]
            num = np.concatenate([ov[:, 2 * i], ov[0:H1, 2 * i + 1]])
            v[r] = num / denom
    return v, res


def kernel(short_perference, current_perference, W, b, current_batch,
           length_input):
    v, _ = run(short_perference, current_perference, W, b, length_input)
    return v.astype(np.float32)
